# revision 25
# baseline (speedup 1.0000x reference)
"""Trainium2 Bass kernel for nn_BERTNet_75256416961146.

Pipeline per sentence (B=64 sentences, sharded 8/core over 8 NeuronCores):
  1. segment-mean of h[b] [512,768] over sorted seg_ids -> means [256,768]
     (computed transposed as means^T [768,256] = h^T @ A, A = one-hot of
     seg_ids). h and A feed the PE as float32r (tf32) so h needs no cast.
  2. P_stack = means @ W1half per token (bf16), scaled by 1/cnt per token,
     quantized to fp8e4 pair-tiles [128, 2, 600].
  3. pre^T [600,1024] via fp8 DoubleRow one-hot gather matmuls (K=256 per
     instruction); tanh(+b1) -> hidT bf16.
  4. logits^T [4,1024] = W2^T @ hid^T (bf16); exp(+b2); 8 tiny PE
     transposes to config-partition layout; softmax normalize; DMA out.

Counts come from 8 tiny PE matmuls (A^T @ ones). DMA traffic is split
across the two HWDGE queues (sync + activation).
"""

import os
import numpy as np
from contextlib import ExitStack

os.environ.setdefault("MYCRO_LOCAL_CACHE", "1")

import concourse.bass as bass
import concourse.tile as tile
from concourse import mybir
from concourse import library_config
from concourse.bass_utils import run_bass_kernel_spmd

# ---- problem shapes (hardcoded per contest rules) ----
B, S, T, C = 64, 512, 256, 1024
D, MLP, CLS = 768, 600, 4
NCORES = 8
BPC = B // NCORES          # sentences per core
P = 128
SCH = S // P               # 4 subtoken chunks
DCH = D // P               # 6 hidden chunks
TCH = T // P               # 2 token chunks
MCH = 5                    # mlp chunks (600 = 4*128 + 88)
NH = C // 512              # 2 config halves for psum tiling
CJ = C // P                # 8 config blocks of 128

f32 = mybir.dt.float32
f32r = mybir.dt.float32r
bf16 = mybir.dt.bfloat16
fp8 = mybir.dt.float8e4
i32 = mybir.dt.int32
Alu = mybir.AluOpType
Act = mybir.ActivationFunctionType
DR = mybir.MatmulPerfMode.DoubleRow

REPEATS = 1

# ---- engine placement knobs ----
GT_DVE = (0, 2)            # which of the 4 GT one-hot writes go on DVE (rest gpsimd)
EVICT_ACT = ()             # means-evict chunks on scalar engine (rest DVE)
PSCALE_DVE = True          # P_stack scale+fp8 quantize on DVE (else ACT)
MEANS_F32R = True          # h/A fed to PE as f32r (no h cast); False = bf16 path
GATHER_FP8_DR = True       # gather via fp8 DoubleRow; False = bf16 matmuls
PHASE = 5                  # debug: 1=head 2=+means/cnt 3=+pstack 4=+gather 5=full
HEAD_A = True              # debug: build A one-hots in head
HEAD_GT = True             # debug: build idx_bc + GT one-hots in head
DMA_ACTQ = True            # seg/conf/out DMAs on the Activation HWDGE queue


def _mrows(m):
    return min(P, MLP - m * P)


def _body(ctx, tc, nc, h_d, seg_d, conf_d, w1_d, b1_d, w2_d, b2_d, out_d,
          iota_row_d, iota_col_d, ident_d):
    const = ctx.enter_context(tc.tile_pool(name="const", bufs=1))
    wstage = ctx.enter_context(tc.tile_pool(name="wstage", bufs=1))

    # ---------- constants ----------
    iota_row = const.tile([P, T], f32r)         # 0..255 along free, all partitions
    nc.sync.dma_start(iota_row[:], iota_row_d.bitcast(f32r))
    iota_col = const.tile([P, 1], f32)          # partition index
    nc.scalar.dma_start(iota_col[:], iota_col_d)
    ident4 = const.tile([CLS, CLS], f32)
    nc.scalar.dma_start(ident4[:], ident_d)
    ones_f = const.tile([P, 2], f32)
    nc.any.memset(ones_f[:], 1.0)
    ones_b = const.tile([P, 2], bf16)
    nc.any.memset(ones_b[:], 1.0)

    # ---------- weights (emitted lazily, after sentence-0's input DMAs) ----------
    wt = {}

    def _load_weights():
        # W1 staged f32 then cast to bf16 [128, k, half*600+n]
        w1st = wstage.tile([P, 2 * DCH, MLP], f32, name="w1st", tag="w1st")
        for c2 in range(2):
            eng = nc.sync if c2 == 0 else nc.scalar
            eng.dma_start(
                w1st[:, c2 * DCH:(c2 + 1) * DCH, :],
                w1_d[c2 * D: (c2 + 1) * D, :].rearrange("(c p) m -> p c m", p=P))
        w1s = const.tile([P, DCH, 2 * MLP], bf16)
        for c in range(2 * DCH):
            half, k = c // DCH, c % DCH
            eng = (nc.vector, nc.scalar, nc.gpsimd)[c % 3]
            if eng is nc.scalar:
                nc.scalar.activation(w1s[:, k, half * MLP:(half + 1) * MLP],
                                     w1st[:, c, :], Act.Copy)
            else:
                eng.tensor_scalar(w1s[:, k, half * MLP:(half + 1) * MLP],
                                  w1st[:, c, :], 1.0, None, Alu.mult)

        w2st = wstage.tile([P, MCH, CLS], f32, name="w2st", tag="w2st")
        nc.vector.memset(w2st[:], 0.0)
        for m in range(MCH):
            rows = _mrows(m)
            nc.scalar.dma_start(w2st[:rows, m, :], w2_d[m * P: m * P + rows, :])
        w2s = const.tile([P, MCH, CLS], bf16)
        nc.vector.tensor_scalar(w2s[:], w2st[:], 1.0, None, Alu.mult)

        b1s = const.tile([P, MCH], f32)
        nc.vector.memset(b1s[:], 0.0)
        for m in range(MCH):
            rows = _mrows(m)
            nc.scalar.dma_start(b1s[:rows, m:m + 1], b1_d[m * P: m * P + rows].unsqueeze(-1))

        b2c = const.tile([CLS, 1], f32)
        nc.scalar.dma_start(b2c[:], b2_d.unsqueeze(-1))
        wt.update(w1s=w1s, w2s=w2s, b1s=b1s, b2c=b2c)

    # ---------- per-sentence pools ----------
    hf_pool = ctx.enter_context(tc.tile_pool(name="hf", bufs=8))
    if not MEANS_F32R:
        hb_pool = ctx.enter_context(tc.tile_pool(name="hb", bufs=8))
    seg_pool = ctx.enter_context(tc.tile_pool(name="segp", bufs=4))
    a_pool = ctx.enter_context(tc.tile_pool(name="ap", bufs=12))
    means_pool = ctx.enter_context(tc.tile_pool(name="meansp", bufs=12))
    recip_pool = ctx.enter_context(tc.tile_pool(name="recipp", bufs=8))
    pstack_pool = ctx.enter_context(tc.tile_pool(name="pstackp", bufs=4))
    conf_pool = ctx.enter_context(tc.tile_pool(name="confp", bufs=4))
    idxbc_pool = ctx.enter_context(tc.tile_pool(name="idxbcp", bufs=4))
    gt_pool = ctx.enter_context(tc.tile_pool(name="gtp", bufs=4))
    hidt_pool = ctx.enter_context(tc.tile_pool(name="hidtp", bufs=2))
    exp_pool = ctx.enter_context(tc.tile_pool(name="expp", bufs=2))
    sm_pool = ctx.enter_context(tc.tile_pool(name="smp", bufs=6))

    ps_means = ctx.enter_context(tc.tile_pool(name="psmeans", bufs=1, space="PSUM"))
    ps_shared = ctx.enter_context(tc.tile_pool(name="psshared", bufs=3, space="PSUM"))
    ps_pstack = ctx.enter_context(tc.tile_pool(name="pspstack", bufs=2, space="PSUM"))

    def _head(b):
        """Input DMAs + A/GT one-hot builds for sentence b, emitted one
        sentence ahead so the DMA/DVE prefetch chain is prioritized."""
        ioq = nc.scalar if DMA_ACTQ else nc.sync
        seg_i = seg_pool.tile([P, SCH], i32, tag="segi", name="seg_i")
        ioq.dma_start(seg_i[:], seg_d[b].rearrange("(q p) -> p q", p=P))
        conf_rows = []
        for half in range(2):
            t = conf_pool.tile([1, C], i32, name=f"confrow{half}", tag="confrow")
            ioq.dma_start(t[:], conf_d[b][:, half].unsqueeze(0))
            conf_rows.append(t)

        hf = []
        for q in range(SCH):
            t = hf_pool.tile([P, D], f32r, name=f"hf{q}", tag="hf")
            nc.sync.dma_start(t[:], h_d[b * S + q * P: b * S + (q + 1) * P, :])
            hf.append(t)
        if not MEANS_F32R:
            hb = []
            for q in range(SCH):
                t = hb_pool.tile([P, D], bf16, name=f"hb{q}", tag="hb")
                nc.vector.tensor_scalar(t[:], hf[q][:].bitcast(f32), 1.0, None,
                                        Alu.mult)
                hb.append(t)
            hf = hb

        A = []
        if HEAD_A:
            seg_f = seg_pool.tile([P, SCH], f32, tag="segf", name="seg_f")
            nc.vector.tensor_scalar(seg_f[:], seg_i[:], 1.0, None, Alu.mult)
            for q in range(SCH):
                t = a_pool.tile([P, T], f32r if MEANS_F32R else bf16,
                                name=f"A{q}", tag="A")
                nc.vector.tensor_scalar(t[:], iota_row[:], seg_f[:, q:q + 1], 0.0,
                                        Alu.subtract, Alu.is_equal)
                A.append(t)

        # conf broadcast + GT one-hot pair tiles [128, 2, C] fp8
        GT = []
        if HEAD_GT:
            idx_bc = []
            for half in range(2):
                t = idxbc_pool.tile([P, C], i32, name=f"idxbc{half}", tag="idxbc")
                nc.gpsimd.partition_broadcast(t[:], conf_rows[half][:])
                idx_bc.append(t)
            for half in range(2):
                t = gt_pool.tile([P, 2, C], fp8 if GATHER_FP8_DR else bf16,
                                 name=f"GT{half}", tag="GT")
                GT.append(t)
            for q in range(SCH):
                half, tq = q // TCH, q % TCH
                eng = nc.vector if q in GT_DVE else nc.gpsimd
                eng.tensor_scalar(GT[half][:, tq, :], idx_bc[half][:], iota_col[:],
                                  float(tq * P + 1), Alu.subtract, Alu.is_equal)
        return dict(hf=hf, A=A, GT=GT)

    dummy_sm = None
    if PHASE < 5:
        dummy_sm = const.tile([P, CJ * CLS], f32)
        nc.any.memset(dummy_sm[:], 0.25)

    def _dummy_out(b):
        (nc.scalar if DMA_ACTQ else nc.sync).dma_start(
            out_d[b * C:(b + 1) * C, :].rearrange("(j p) k -> p j k", p=P),
            dummy_sm[:].rearrange("p (j k) -> p j k", k=CLS))

    blist = [bb for _ in range(REPEATS) for bb in range(BPC)]
    heads = {0: _head(blist[0])}
    for bi, b in enumerate(blist):
        if bi not in heads:
            heads[bi] = _head(blist[bi])
        st_h = heads.pop(bi)
        hf, A, GT = st_h["hf"], st_h["A"], st_h["GT"]
        if PHASE < 2:
            if bi + 1 < len(blist):
                heads[bi + 1] = _head(blist[bi + 1])
            _dummy_out(b)
            continue

        # ---- means^T [768, 256] = h^T @ A (f32r, 6 m-chunks) ----
        means = []
        for m in range(DCH):
            mps = ps_means.tile([P, 512], f32, name="mps", tag="mps")[:, 0:T]
            for q in range(SCH):
                nc.tensor.matmul(mps[:], hf[q][:, m * P:(m + 1) * P], A[q][:],
                                 start=(q == 0), stop=(q == SCH - 1))
            msb = means_pool.tile([P, T], bf16, name=f"means{m}", tag="means")
            if m in EVICT_ACT:
                nc.scalar.activation(msb[:], mps[:], Act.Copy)
            else:
                nc.vector.tensor_scalar(msb[:], mps[:], 1.0, None, Alu.mult)
            means.append(msb)

        # ---- counts via PE (A^T @ ones), then reciprocal ----
        recips = []
        for mt in range(TCH):
            cps = ps_shared.tile([P, 512], f32, name="cps", tag="shared")[:, 0:2]
            for q in range(SCH):
                nc.tensor.matmul(cps[:], A[q][:, mt * P:(mt + 1) * P],
                                 ones_f[:].bitcast(f32r) if MEANS_F32R else ones_b[:],
                                 start=(q == 0), stop=(q == SCH - 1))
            csb = recip_pool.tile([P, 1], f32, name="cnt", tag="cnt", bufs=4)
            nc.vector.tensor_scalar(csb[:], cps[:, 0:1], 1.0, None, Alu.max)
            r = recip_pool.tile([P, 1], f32, name="recip", tag="recip", bufs=8)
            nc.vector.reciprocal(r[:], csb[:])
            recips.append(r)

        if PHASE < 3:
            if bi + 1 < len(blist):
                heads[bi + 1] = _head(blist[bi + 1])
            _dummy_out(b)
            continue

        if not wt:
            _load_weights()
        w1s, w2s, b1s, b2c = wt["w1s"], wt["w2s"], wt["b1s"], wt["b2c"]

        # ---- P_stack pair tiles [128, 2, 600] fp8, scaled by 1/cnt ----
        pstack = [pstack_pool.tile([P, TCH, 640], fp8 if GATHER_FP8_DR else bf16,
                                   name=f"pstack{h}", tag="pstack")
                  for h in range(2)]
        for mq in range(SCH):
            half, tq = mq // TCH, mq % TCH
            pps = ps_pstack.tile([P, 640], f32, name="pps", tag="pps")[:, 0:MLP]
            for ns, ne in ((0, 512), (512, MLP)):
                for k in range(DCH):
                    nc.tensor.matmul(pps[:, ns:ne],
                                     means[k][:, tq * P:(tq + 1) * P],
                                     w1s[:, k, half * MLP + ns: half * MLP + ne],
                                     start=(k == 0), stop=(k == DCH - 1))
            if PSCALE_DVE:
                nc.vector.tensor_scalar(pstack[half][:, tq, 0:MLP], pps[:],
                                        recips[tq][:], None, Alu.mult)
            else:
                nc.scalar.activation(pstack[half][:, tq, 0:MLP], pps[:],
                                     Act.Copy, scale=recips[tq][:])

        if bi + 1 < len(blist):
            heads[bi + 1] = _head(blist[bi + 1])
        if PHASE < 4:
            _dummy_out(b)
            continue

        # ---- pre^T via fp8 DoubleRow gather; tanh -> hid^T bf16; then
        # logits^T = W2^T @ hid^T and exp(+b2), per config-half so the
        # logits matmuls of half 0 overlap the gather of half 1 ----
        hidT = hidt_pool.tile([P, MCH, C], bf16)
        exp_sb = exp_pool.tile([CLS, C], f32)
        for n2 in range(NH):
            for m in range(MCH):
                rows = _mrows(m)
                pre = ps_shared.tile([P, 512], f32, name="pre", tag="shared")
                if GATHER_FP8_DR:
                    for half in range(2):
                        nc.tensor.matmul(pre[:rows],
                                         pstack[half][:, :, m * P:m * P + rows],
                                         GT[half][:, :, n2 * 512:(n2 + 1) * 512],
                                         start=(half == 0), stop=(half == 1),
                                         perf_mode=DR)
                else:
                    for half in range(2):
                        for k2 in range(2):
                            nc.tensor.matmul(
                                pre[:rows],
                                pstack[half][:, k2, m * P:m * P + rows],
                                GT[half][:, k2, n2 * 512:(n2 + 1) * 512],
                                start=(half == 0 and k2 == 0),
                                stop=(half == 1 and k2 == 1))
                nc.scalar.activation(hidT[:rows, m, n2 * 512:(n2 + 1) * 512],
                                     pre[:rows], Act.Tanh, bias=b1s[:rows, m:m + 1])
            if PHASE < 5:
                continue
            lg = ps_shared.tile([P, 512], f32, name="lg", tag="shared")
            for m in range(MCH):
                rows = _mrows(m)
                nc.tensor.matmul(lg[:CLS], w2s[:rows, m, :],
                                 hidT[:rows, m, n2 * 512:(n2 + 1) * 512],
                                 start=(m == 0), stop=(m == MCH - 1))
            nc.scalar.activation(exp_sb[:, n2 * 512:(n2 + 1) * 512], lg[:CLS],
                                 Act.Exp, bias=b2c[:])

        if PHASE < 5:
            _dummy_out(b)
            continue

        # ---- PE-transpose exp^T -> [config, class]; normalize; DMA out ----
        expT = ps_shared.tile([P, 512], f32, name="expT", tag="shared")[:, 0:CJ * CLS]
        for j in range(CJ):
            nc.tensor.transpose(expT[:, j * CLS:(j + 1) * CLS],
                                exp_sb[:, j * P:(j + 1) * P], ident4[:])
        den = sm_pool.tile([P, CJ], f32, name="den", tag="den", bufs=2)
        nc.vector.tensor_reduce(den[:], expT[:].rearrange("p (j k) -> p j k", k=CLS),
                                mybir.AxisListType.X, Alu.add)
        rden = sm_pool.tile([P, CJ], f32, name="rden", tag="rden", bufs=2)
        nc.vector.reciprocal(rden[:], den[:])
        sm = sm_pool.tile([P, CJ * CLS], f32, name="sm", tag="sm", bufs=2)
        nc.vector.tensor_tensor(sm[:].rearrange("p (j k) -> p j k", k=CLS),
                                expT[:].rearrange("p (j k) -> p j k", k=CLS),
                                rden[:].unsqueeze(-1).broadcast_to((P, CJ, CLS)),
                                Alu.mult)
        (nc.scalar if DMA_ACTQ else nc.sync).dma_start(
            out_d[b * C:(b + 1) * C, :].rearrange("(j p) k -> p j k", p=P),
            sm[:].rearrange("p (j k) -> p j k", k=CLS))


def build_module():
    nc = bass.Bass("TRN2", target_bir_lowering=False, debug=False)

    h_d = nc.dram_tensor("h", [BPC * S, D], f32r, kind="ExternalInput").ap()
    seg_d = nc.dram_tensor("seg", [BPC, S], i32, kind="ExternalInput").ap()
    conf_d = nc.dram_tensor("conf", [BPC, C, 2], i32, kind="ExternalInput").ap()
    w1_d = nc.dram_tensor("w1", [2 * D, MLP], f32, kind="ExternalInput").ap()
    b1_d = nc.dram_tensor("b1", [MLP], f32, kind="ExternalInput").ap()
    w2_d = nc.dram_tensor("w2", [MLP, CLS], f32, kind="ExternalInput").ap()
    b2_d = nc.dram_tensor("b2", [CLS], f32, kind="ExternalInput").ap()
    out_d = nc.dram_tensor("out", [BPC * C, CLS], f32, kind="ExternalOutput").ap()

    iota_row_d = nc.inline_tensor(
        np.broadcast_to(np.arange(T, dtype=np.float32), (P, T)).copy(), "c_iota_row").ap()
    iota_col_d = nc.inline_tensor(
        np.arange(P, dtype=np.float32).reshape(P, 1), "c_iota_col").ap()
    ident_d = nc.inline_tensor(np.eye(CLS, dtype=np.float32), "c_ident").ap()

    with tile.TileContext(nc) as tc:
        with ExitStack() as ctx:
            nc.gpsimd.load_library(library_config.mlp)
            _body(ctx, tc, nc, h_d, seg_d, conf_d, w1_d, b1_d, w2_d, b2_d, out_d,
                  iota_row_d, iota_col_d, ident_d)
    # Raw Bass skips several Bacc.compile() passes the NEFF compiler needs:
    # - move_matmul_waits_to_ldweights + generate_event_semaphores: TRN2 allows
    #   at most 1 sync wait per instruction ("Too many sync wait commands")
    # - codegen_inst_isa_subclasses: fills .instr bytes for the gpsimd
    #   extended-ISA ops ("ISA wrong length")
    import bass_rust as _bass_rust
    _bass_rust.move_matmul_waits_to_ldweights(nc.m)
    _bass_rust.generate_event_semaphores(nc)
    mybir.codegen_inst_isa_subclasses(nc)
    return nc


_NC = None


def _get_nc():
    global _NC
    if _NC is None:
        _NC = build_module()
    return _NC


_RUNNER = None


def _get_runner():
    """Build the jitted PJRT callable once per process (run_bass_kernel_spmd
    retraces jax on every call, which costs seconds)."""
    global _RUNNER
    if _RUNNER is not None:
        return _RUNNER
    import jax
    from jax.sharding import Mesh, PartitionSpec
    from jax.experimental.shard_map import shard_map
    from concourse import bass2jax

    nc = _get_nc()
    bass2jax.install_neuronx_cc_hook()
    partition_name = nc.partition_id_tensor.name if nc.partition_id_tensor else None
    in_names, out_names, out_avals, out_shapes = [], [], [], []
    for alloc in nc.m.functions[0].allocations:
        if not isinstance(alloc, mybir.MemoryLocationSet):
            continue
        name = alloc.memorylocations[0].name
        if alloc.kind == "ExternalInput":
            if name != partition_name:
                in_names.append(name)
        elif alloc.kind == "ExternalOutput":
            shape = tuple(alloc.tensor_shape)
            dtype = mybir.dt.np(alloc.dtype)
            out_avals.append(jax.core.ShapedArray(shape, dtype))
            out_names.append(name)
            out_shapes.append((shape, dtype))
    all_in_names = list(in_names) + list(out_names)
    if partition_name is not None:
        all_in_names.append(partition_name)

    def _pjrt_body(*args):
        operands = list(args)
        if partition_name is not None:
            operands.append(bass2jax.partition_id_tensor())
        return tuple(bass2jax._bass_exec_p.bind(
            *operands,
            out_avals=tuple(out_avals),
            in_names=tuple(all_in_names),
            out_names=tuple(out_names),
            lowering_input_output_aliases=(),
            sim_require_finite=True,
            sim_require_nnan=True,
            nc=nc,
        ))

    devices = jax.devices()[:NCORES]
    mesh = Mesh(np.asarray(devices), ("core",))
    n_outs = len(out_names)
    in_specs = (PartitionSpec("core"),) * (len(in_names) + n_outs)
    out_specs = (PartitionSpec("core"),) * n_outs
    fn = jax.jit(shard_map(_pjrt_body, mesh=mesh, in_specs=in_specs,
                           out_specs=out_specs, check_rep=False),
                 keep_unused=True)
    _RUNNER = (fn, in_names, out_names, out_shapes)
    return _RUNNER


def run_cached(in_maps):
    """Execute via the cached jit; returns list of per-core {name: np.ndarray}."""
    fn, in_names, out_names, out_shapes = _get_runner()
    concat_in = [np.concatenate([in_maps[c][n] for c in range(NCORES)], axis=0)
                 for n in in_names]
    concat_zeros = [np.zeros((NCORES * s[0], *s[1:]), dt)
                    for (s, dt) in out_shapes]
    out_arrs = fn(*concat_in, *concat_zeros)
    res = []
    for c in range(NCORES):
        res.append({name: np.asarray(out_arrs[i]).reshape(
            NCORES, *out_shapes[i][0])[c] for i, name in enumerate(out_names)})
    return res


def make_in_maps(h, seg_ids, conf, W1, b1, W2, b2):
    h = np.ascontiguousarray(np.asarray(h), dtype=np.float32)
    seg_ids = np.ascontiguousarray(np.asarray(seg_ids), dtype=np.int32)
    conf = np.ascontiguousarray(np.asarray(conf), dtype=np.int32)
    W1 = np.ascontiguousarray(np.asarray(W1), dtype=np.float32)
    b1 = np.ascontiguousarray(np.asarray(b1), dtype=np.float32)
    W2 = np.ascontiguousarray(np.asarray(W2), dtype=np.float32)
    b2 = np.ascontiguousarray(np.asarray(b2), dtype=np.float32)
    in_maps = []
    for i in range(NCORES):
        sl = slice(i * BPC, (i + 1) * BPC)
        in_maps.append({
            "h": h[sl].reshape(BPC * S, D),
            "seg": seg_ids[sl],
            "conf": conf[sl],
            "w1": W1, "b1": b1, "w2": W2, "b2": b2,
        })
    return in_maps


def run(in_maps, trace=False, **kwargs):
    nc = _get_nc()
    return run_bass_kernel_spmd(nc, in_maps, core_ids=list(range(NCORES)),
                                trace=trace, **kwargs)


def kernel(h, seg_ids, conf, W1, b1, W2, b2):
    global _RUNNER
    in_maps = make_in_maps(h, seg_ids, conf, W1, b1, W2, b2)
    # The axon-tunneled devices occasionally fail the first execution after a
    # fresh NEFF load (NRT_EXEC_UNIT_UNRECOVERABLE); a retry on a rebuilt
    # executable has always succeeded. Guard the graded call.
    last = None
    for attempt in range(3):
        try:
            res = run_cached(in_maps)
            break
        except Exception as e:  # noqa: BLE001
            last = e
            _RUNNER = None
            import time as _time
            _time.sleep(2.0 * (attempt + 1))
    else:
        raise last
    outs = [res[i]["out"] for i in range(NCORES)]
    return np.concatenate(outs, axis=0)


# revision 26
# speedup vs baseline: 2.0765x; 2.0765x over previous
"""Trainium2 Bass kernel for nn_BERTNet_75256416961146.

Pipeline per sentence (B=64 sentences, sharded 8/core over 8 NeuronCores):
  1. segment-mean of h[b] [512,768] over sorted seg_ids -> means [256,768]
     (computed transposed as means^T [768,256] = h^T @ A, A = one-hot of
     seg_ids). h and A feed the PE as float32r (tf32) so h needs no cast.
  2. P_stack = means @ W1half per token (bf16), scaled by 1/cnt per token,
     quantized to fp8e4 pair-tiles [128, 2, 600].
  3. pre^T [600,1024] via fp8 DoubleRow one-hot gather matmuls (K=256 per
     instruction); tanh(+b1) -> hidT bf16.
  4. logits^T [4,1024] = W2^T @ hid^T (bf16); exp(+b2); 8 tiny PE
     transposes to config-partition layout; softmax normalize; DMA out.

Counts come from 8 tiny PE matmuls (A^T @ ones). DMA traffic is split
across the two HWDGE queues (sync + activation).
"""

import os
import numpy as np
from contextlib import ExitStack

os.environ.setdefault("MYCRO_LOCAL_CACHE", "1")

import concourse.bass as bass
import concourse.tile as tile
from concourse import mybir
from concourse import library_config
from concourse.bass_utils import run_bass_kernel_spmd

# ---- problem shapes (hardcoded per contest rules) ----
B, S, T, C = 64, 512, 256, 1024
D, MLP, CLS = 768, 600, 4
NCORES = 8
BPC = B // NCORES          # sentences per core
P = 128
SCH = S // P               # 4 subtoken chunks
DCH = D // P               # 6 hidden chunks
TCH = T // P               # 2 token chunks
MCH = 5                    # mlp chunks (600 = 4*128 + 88)
NH = C // 512              # 2 config halves for psum tiling
CJ = C // P                # 8 config blocks of 128

f32 = mybir.dt.float32
f32r = mybir.dt.float32r
bf16 = mybir.dt.bfloat16
fp8 = mybir.dt.float8e4
i32 = mybir.dt.int32
Alu = mybir.AluOpType
Act = mybir.ActivationFunctionType
DR = mybir.MatmulPerfMode.DoubleRow

REPEATS = 1

# ---- engine placement knobs ----
GT_DVE = (0, 1, 2, 3)      # GT one-hot builds on DVE (gpsimd tensor_scalar is ~13us/op on HW)
EVICT_ACT = ()             # means-evict chunks on scalar engine (rest DVE)
PSCALE_DVE = True          # P_stack scale+fp8 quantize on DVE (else ACT)
MEANS_F32R = True          # h/A fed to PE as f32r (no h cast); False = bf16 path
GATHER_FP8_DR = True       # gather via fp8 DoubleRow; False = bf16 matmuls
PHASE = 5                  # debug: 1=head 2=+means/cnt 3=+pstack 4=+gather 5=full
HEAD_A = True              # debug: build A one-hots in head
HEAD_GT = True             # debug: build idx_bc + GT one-hots in head
DMA_ACTQ = True            # seg/conf/out DMAs on the Activation HWDGE queue


def _mrows(m):
    return min(P, MLP - m * P)


def _body(ctx, tc, nc, h_d, seg_d, conf_d, w1_d, b1_d, w2_d, b2_d, out_d,
          iota_row_d, iota_col_d, ident_d):
    const = ctx.enter_context(tc.tile_pool(name="const", bufs=1))
    wstage = ctx.enter_context(tc.tile_pool(name="wstage", bufs=1))

    # ---------- constants ----------
    iota_row = const.tile([P, T], f32r)         # 0..255 along free, all partitions
    nc.sync.dma_start(iota_row[:], iota_row_d.bitcast(f32r))
    iota_col = const.tile([P, 1], f32)          # partition index
    nc.scalar.dma_start(iota_col[:], iota_col_d)
    ident4 = const.tile([CLS, CLS], f32)
    nc.scalar.dma_start(ident4[:], ident_d)
    ones_f = const.tile([P, 2], f32)
    nc.any.memset(ones_f[:], 1.0)
    ones_b = const.tile([P, 2], bf16)
    nc.any.memset(ones_b[:], 1.0)

    # ---------- weights (emitted lazily, after sentence-0's input DMAs) ----------
    wt = {}

    def _load_weights():
        # W1 staged f32 then cast to bf16 [128, k, half*600+n]
        w1st = wstage.tile([P, 2 * DCH, MLP], f32, name="w1st", tag="w1st")
        for c2 in range(2):
            eng = nc.sync if c2 == 0 else nc.scalar
            eng.dma_start(
                w1st[:, c2 * DCH:(c2 + 1) * DCH, :],
                w1_d[c2 * D: (c2 + 1) * D, :].rearrange("(c p) m -> p c m", p=P))
        w1s = const.tile([P, DCH, 2 * MLP], bf16)
        for c in range(2 * DCH):
            half, k = c // DCH, c % DCH
            eng = (nc.vector, nc.scalar)[c % 2]
            if eng is nc.scalar:
                nc.scalar.activation(w1s[:, k, half * MLP:(half + 1) * MLP],
                                     w1st[:, c, :], Act.Copy)
            else:
                eng.tensor_scalar(w1s[:, k, half * MLP:(half + 1) * MLP],
                                  w1st[:, c, :], 1.0, None, Alu.mult)

        w2st = wstage.tile([P, MCH, CLS], f32, name="w2st", tag="w2st")
        nc.vector.memset(w2st[:], 0.0)
        for m in range(MCH):
            rows = _mrows(m)
            nc.scalar.dma_start(w2st[:rows, m, :], w2_d[m * P: m * P + rows, :])
        w2s = const.tile([P, MCH, CLS], bf16)
        nc.vector.tensor_scalar(w2s[:], w2st[:], 1.0, None, Alu.mult)

        b1s = const.tile([P, MCH], f32)
        nc.vector.memset(b1s[:], 0.0)
        for m in range(MCH):
            rows = _mrows(m)
            nc.scalar.dma_start(b1s[:rows, m:m + 1], b1_d[m * P: m * P + rows].unsqueeze(-1))

        b2c = const.tile([CLS, 1], f32)
        nc.scalar.dma_start(b2c[:], b2_d.unsqueeze(-1))
        wt.update(w1s=w1s, w2s=w2s, b1s=b1s, b2c=b2c)

    # ---------- per-sentence pools ----------
    hf_pool = ctx.enter_context(tc.tile_pool(name="hf", bufs=8))
    if not MEANS_F32R:
        hb_pool = ctx.enter_context(tc.tile_pool(name="hb", bufs=8))
    seg_pool = ctx.enter_context(tc.tile_pool(name="segp", bufs=4))
    a_pool = ctx.enter_context(tc.tile_pool(name="ap", bufs=12))
    means_pool = ctx.enter_context(tc.tile_pool(name="meansp", bufs=12))
    recip_pool = ctx.enter_context(tc.tile_pool(name="recipp", bufs=8))
    pstack_pool = ctx.enter_context(tc.tile_pool(name="pstackp", bufs=4))
    conf_pool = ctx.enter_context(tc.tile_pool(name="confp", bufs=4))
    idxbc_pool = ctx.enter_context(tc.tile_pool(name="idxbcp", bufs=4))
    gt_pool = ctx.enter_context(tc.tile_pool(name="gtp", bufs=4))
    hidt_pool = ctx.enter_context(tc.tile_pool(name="hidtp", bufs=2))
    exp_pool = ctx.enter_context(tc.tile_pool(name="expp", bufs=2))
    sm_pool = ctx.enter_context(tc.tile_pool(name="smp", bufs=6))

    ps_means = ctx.enter_context(tc.tile_pool(name="psmeans", bufs=1, space="PSUM"))
    ps_shared = ctx.enter_context(tc.tile_pool(name="psshared", bufs=3, space="PSUM"))
    ps_pstack = ctx.enter_context(tc.tile_pool(name="pspstack", bufs=2, space="PSUM"))

    def _head(b):
        """Input DMAs + A/GT one-hot builds for sentence b, emitted one
        sentence ahead so the DMA/DVE prefetch chain is prioritized."""
        ioq = nc.scalar if DMA_ACTQ else nc.sync
        seg_i = seg_pool.tile([P, SCH], i32, tag="segi", name="seg_i")
        ioq.dma_start(seg_i[:], seg_d[b].rearrange("(q p) -> p q", p=P))
        conf_rows = []
        for half in range(2):
            t = conf_pool.tile([1, C], i32, name=f"confrow{half}", tag="confrow")
            ioq.dma_start(t[:], conf_d[b][:, half].unsqueeze(0))
            conf_rows.append(t)

        hf = []
        for q in range(SCH):
            t = hf_pool.tile([P, D], f32r, name=f"hf{q}", tag="hf")
            nc.sync.dma_start(t[:], h_d[b * S + q * P: b * S + (q + 1) * P, :])
            hf.append(t)
        if not MEANS_F32R:
            hb = []
            for q in range(SCH):
                t = hb_pool.tile([P, D], bf16, name=f"hb{q}", tag="hb")
                nc.vector.tensor_scalar(t[:], hf[q][:].bitcast(f32), 1.0, None,
                                        Alu.mult)
                hb.append(t)
            hf = hb

        A = []
        if HEAD_A:
            seg_f = seg_pool.tile([P, SCH], f32, tag="segf", name="seg_f")
            nc.vector.tensor_scalar(seg_f[:], seg_i[:], 1.0, None, Alu.mult)
            for q in range(SCH):
                t = a_pool.tile([P, T], f32r if MEANS_F32R else bf16,
                                name=f"A{q}", tag="A")
                nc.vector.tensor_scalar(t[:], iota_row[:], seg_f[:, q:q + 1], 0.0,
                                        Alu.subtract, Alu.is_equal)
                A.append(t)

        # conf broadcast + GT one-hot pair tiles [128, 2, C] fp8
        GT = []
        if HEAD_GT:
            idx_bc = []
            for half in range(2):
                t = idxbc_pool.tile([P, C], i32, name=f"idxbc{half}", tag="idxbc")
                nc.gpsimd.partition_broadcast(t[:], conf_rows[half][:])
                idx_bc.append(t)
            for half in range(2):
                t = gt_pool.tile([P, 2, C], fp8 if GATHER_FP8_DR else bf16,
                                 name=f"GT{half}", tag="GT")
                GT.append(t)
            for q in range(SCH):
                half, tq = q // TCH, q % TCH
                eng = nc.vector if q in GT_DVE else nc.gpsimd
                eng.tensor_scalar(GT[half][:, tq, :], idx_bc[half][:], iota_col[:],
                                  float(tq * P + 1), Alu.subtract, Alu.is_equal)
        return dict(hf=hf, A=A, GT=GT)

    dummy_sm = None
    if PHASE < 5:
        dummy_sm = const.tile([P, CJ * CLS], f32)
        nc.any.memset(dummy_sm[:], 0.25)

    def _dummy_out(b):
        (nc.scalar if DMA_ACTQ else nc.sync).dma_start(
            out_d[b * C:(b + 1) * C, :].rearrange("(j p) k -> p j k", p=P),
            dummy_sm[:].rearrange("p (j k) -> p j k", k=CLS))

    blist = [bb for _ in range(REPEATS) for bb in range(BPC)]
    heads = {0: _head(blist[0])}
    for bi, b in enumerate(blist):
        if bi not in heads:
            heads[bi] = _head(blist[bi])
        st_h = heads.pop(bi)
        hf, A, GT = st_h["hf"], st_h["A"], st_h["GT"]
        if PHASE < 2:
            if bi + 1 < len(blist):
                heads[bi + 1] = _head(blist[bi + 1])
            _dummy_out(b)
            continue

        # ---- means^T [768, 256] = h^T @ A (f32r, 6 m-chunks) ----
        means = []
        for m in range(DCH):
            mps = ps_means.tile([P, 512], f32, name="mps", tag="mps")[:, 0:T]
            for q in range(SCH):
                nc.tensor.matmul(mps[:], hf[q][:, m * P:(m + 1) * P], A[q][:],
                                 start=(q == 0), stop=(q == SCH - 1))
            msb = means_pool.tile([P, T], bf16, name=f"means{m}", tag="means")
            if m in EVICT_ACT:
                nc.scalar.activation(msb[:], mps[:], Act.Copy)
            else:
                nc.vector.tensor_scalar(msb[:], mps[:], 1.0, None, Alu.mult)
            means.append(msb)

        # ---- counts via PE (A^T @ ones), then reciprocal ----
        recips = []
        for mt in range(TCH):
            cps = ps_shared.tile([P, 512], f32, name="cps", tag="shared")[:, 0:2]
            for q in range(SCH):
                nc.tensor.matmul(cps[:], A[q][:, mt * P:(mt + 1) * P],
                                 ones_f[:].bitcast(f32r) if MEANS_F32R else ones_b[:],
                                 start=(q == 0), stop=(q == SCH - 1))
            csb = recip_pool.tile([P, 1], f32, name="cnt", tag="cnt", bufs=4)
            nc.vector.tensor_scalar(csb[:], cps[:, 0:1], 1.0, None, Alu.max)
            r = recip_pool.tile([P, 1], f32, name="recip", tag="recip", bufs=8)
            nc.vector.reciprocal(r[:], csb[:])
            recips.append(r)

        if PHASE < 3:
            if bi + 1 < len(blist):
                heads[bi + 1] = _head(blist[bi + 1])
            _dummy_out(b)
            continue

        if not wt:
            _load_weights()
        w1s, w2s, b1s, b2c = wt["w1s"], wt["w2s"], wt["b1s"], wt["b2c"]

        # ---- P_stack pair tiles [128, 2, 600] fp8, scaled by 1/cnt ----
        pstack = [pstack_pool.tile([P, TCH, 640], fp8 if GATHER_FP8_DR else bf16,
                                   name=f"pstack{h}", tag="pstack")
                  for h in range(2)]
        for mq in range(SCH):
            half, tq = mq // TCH, mq % TCH
            pps = ps_pstack.tile([P, 640], f32, name="pps", tag="pps")[:, 0:MLP]
            for ns, ne in ((0, 512), (512, MLP)):
                for k in range(DCH):
                    nc.tensor.matmul(pps[:, ns:ne],
                                     means[k][:, tq * P:(tq + 1) * P],
                                     w1s[:, k, half * MLP + ns: half * MLP + ne],
                                     start=(k == 0), stop=(k == DCH - 1))
            if PSCALE_DVE:
                nc.vector.tensor_scalar(pstack[half][:, tq, 0:MLP], pps[:],
                                        recips[tq][:], None, Alu.mult)
            else:
                nc.scalar.activation(pstack[half][:, tq, 0:MLP], pps[:],
                                     Act.Copy, scale=recips[tq][:])

        if bi + 1 < len(blist):
            heads[bi + 1] = _head(blist[bi + 1])
        if PHASE < 4:
            _dummy_out(b)
            continue

        # ---- pre^T via fp8 DoubleRow gather; tanh -> hid^T bf16; then
        # logits^T = W2^T @ hid^T and exp(+b2), per config-half so the
        # logits matmuls of half 0 overlap the gather of half 1 ----
        hidT = hidt_pool.tile([P, MCH, C], bf16)
        exp_sb = exp_pool.tile([CLS, C], f32)
        for n2 in range(NH):
            for m in range(MCH):
                rows = _mrows(m)
                pre = ps_shared.tile([P, 512], f32, name="pre", tag="shared")
                if GATHER_FP8_DR:
                    for half in range(2):
                        nc.tensor.matmul(pre[:rows],
                                         pstack[half][:, :, m * P:m * P + rows],
                                         GT[half][:, :, n2 * 512:(n2 + 1) * 512],
                                         start=(half == 0), stop=(half == 1),
                                         perf_mode=DR)
                else:
                    for half in range(2):
                        for k2 in range(2):
                            nc.tensor.matmul(
                                pre[:rows],
                                pstack[half][:, k2, m * P:m * P + rows],
                                GT[half][:, k2, n2 * 512:(n2 + 1) * 512],
                                start=(half == 0 and k2 == 0),
                                stop=(half == 1 and k2 == 1))
                nc.scalar.activation(hidT[:rows, m, n2 * 512:(n2 + 1) * 512],
                                     pre[:rows], Act.Tanh, bias=b1s[:rows, m:m + 1])
            if PHASE < 5:
                continue
            lg = ps_shared.tile([P, 512], f32, name="lg", tag="shared")
            for m in range(MCH):
                rows = _mrows(m)
                nc.tensor.matmul(lg[:CLS], w2s[:rows, m, :],
                                 hidT[:rows, m, n2 * 512:(n2 + 1) * 512],
                                 start=(m == 0), stop=(m == MCH - 1))
            nc.scalar.activation(exp_sb[:, n2 * 512:(n2 + 1) * 512], lg[:CLS],
                                 Act.Exp, bias=b2c[:])

        if PHASE < 5:
            _dummy_out(b)
            continue

        # ---- PE-transpose exp^T -> [config, class]; normalize; DMA out ----
        expT = ps_shared.tile([P, 512], f32, name="expT", tag="shared")[:, 0:CJ * CLS]
        for j in range(CJ):
            nc.tensor.transpose(expT[:, j * CLS:(j + 1) * CLS],
                                exp_sb[:, j * P:(j + 1) * P], ident4[:])
        den = sm_pool.tile([P, CJ], f32, name="den", tag="den", bufs=2)
        nc.vector.tensor_reduce(den[:], expT[:].rearrange("p (j k) -> p j k", k=CLS),
                                mybir.AxisListType.X, Alu.add)
        rden = sm_pool.tile([P, CJ], f32, name="rden", tag="rden", bufs=2)
        nc.vector.reciprocal(rden[:], den[:])
        sm = sm_pool.tile([P, CJ * CLS], f32, name="sm", tag="sm", bufs=2)
        nc.vector.tensor_tensor(sm[:].rearrange("p (j k) -> p j k", k=CLS),
                                expT[:].rearrange("p (j k) -> p j k", k=CLS),
                                rden[:].unsqueeze(-1).broadcast_to((P, CJ, CLS)),
                                Alu.mult)
        (nc.scalar if DMA_ACTQ else nc.sync).dma_start(
            out_d[b * C:(b + 1) * C, :].rearrange("(j p) k -> p j k", p=P),
            sm[:].rearrange("p (j k) -> p j k", k=CLS))


def build_module():
    nc = bass.Bass("TRN2", target_bir_lowering=False, debug=False)

    h_d = nc.dram_tensor("h", [BPC * S, D], f32r, kind="ExternalInput").ap()
    seg_d = nc.dram_tensor("seg", [BPC, S], i32, kind="ExternalInput").ap()
    conf_d = nc.dram_tensor("conf", [BPC, C, 2], i32, kind="ExternalInput").ap()
    w1_d = nc.dram_tensor("w1", [2 * D, MLP], f32, kind="ExternalInput").ap()
    b1_d = nc.dram_tensor("b1", [MLP], f32, kind="ExternalInput").ap()
    w2_d = nc.dram_tensor("w2", [MLP, CLS], f32, kind="ExternalInput").ap()
    b2_d = nc.dram_tensor("b2", [CLS], f32, kind="ExternalInput").ap()
    out_d = nc.dram_tensor("out", [BPC * C, CLS], f32, kind="ExternalOutput").ap()

    iota_row_d = nc.inline_tensor(
        np.broadcast_to(np.arange(T, dtype=np.float32), (P, T)).copy(), "c_iota_row").ap()
    iota_col_d = nc.inline_tensor(
        np.arange(P, dtype=np.float32).reshape(P, 1), "c_iota_col").ap()
    ident_d = nc.inline_tensor(np.eye(CLS, dtype=np.float32), "c_ident").ap()

    with tile.TileContext(nc) as tc:
        with ExitStack() as ctx:
            nc.gpsimd.load_library(library_config.mlp)
            _body(ctx, tc, nc, h_d, seg_d, conf_d, w1_d, b1_d, w2_d, b2_d, out_d,
                  iota_row_d, iota_col_d, ident_d)
    # Raw Bass skips several Bacc.compile() passes the NEFF compiler needs:
    # - move_matmul_waits_to_ldweights + generate_event_semaphores: TRN2 allows
    #   at most 1 sync wait per instruction ("Too many sync wait commands")
    # - codegen_inst_isa_subclasses: fills .instr bytes for the gpsimd
    #   extended-ISA ops ("ISA wrong length")
    import bass_rust as _bass_rust
    _bass_rust.move_matmul_waits_to_ldweights(nc.m)
    _bass_rust.generate_event_semaphores(nc)
    mybir.codegen_inst_isa_subclasses(nc)
    return nc


_NC = None


def _get_nc():
    global _NC
    if _NC is None:
        _NC = build_module()
    return _NC


_RUNNER = None


def _get_runner():
    """Build the jitted PJRT callable once per process (run_bass_kernel_spmd
    retraces jax on every call, which costs seconds)."""
    global _RUNNER
    if _RUNNER is not None:
        return _RUNNER
    import jax
    from jax.sharding import Mesh, PartitionSpec
    from jax.experimental.shard_map import shard_map
    from concourse import bass2jax

    nc = _get_nc()
    bass2jax.install_neuronx_cc_hook()
    partition_name = nc.partition_id_tensor.name if nc.partition_id_tensor else None
    in_names, out_names, out_avals, out_shapes = [], [], [], []
    for alloc in nc.m.functions[0].allocations:
        if not isinstance(alloc, mybir.MemoryLocationSet):
            continue
        name = alloc.memorylocations[0].name
        if alloc.kind == "ExternalInput":
            if name != partition_name:
                in_names.append(name)
        elif alloc.kind == "ExternalOutput":
            shape = tuple(alloc.tensor_shape)
            dtype = mybir.dt.np(alloc.dtype)
            out_avals.append(jax.core.ShapedArray(shape, dtype))
            out_names.append(name)
            out_shapes.append((shape, dtype))
    all_in_names = list(in_names) + list(out_names)
    if partition_name is not None:
        all_in_names.append(partition_name)

    def _pjrt_body(*args):
        operands = list(args)
        if partition_name is not None:
            operands.append(bass2jax.partition_id_tensor())
        return tuple(bass2jax._bass_exec_p.bind(
            *operands,
            out_avals=tuple(out_avals),
            in_names=tuple(all_in_names),
            out_names=tuple(out_names),
            lowering_input_output_aliases=(),
            sim_require_finite=True,
            sim_require_nnan=True,
            nc=nc,
        ))

    devices = jax.devices()[:NCORES]
    mesh = Mesh(np.asarray(devices), ("core",))
    n_outs = len(out_names)
    in_specs = (PartitionSpec("core"),) * (len(in_names) + n_outs)
    out_specs = (PartitionSpec("core"),) * n_outs
    fn = jax.jit(shard_map(_pjrt_body, mesh=mesh, in_specs=in_specs,
                           out_specs=out_specs, check_rep=False),
                 keep_unused=True)
    _RUNNER = (fn, in_names, out_names, out_shapes)
    return _RUNNER


def run_cached(in_maps):
    """Execute via the cached jit; returns list of per-core {name: np.ndarray}."""
    fn, in_names, out_names, out_shapes = _get_runner()
    concat_in = [np.concatenate([in_maps[c][n] for c in range(NCORES)], axis=0)
                 for n in in_names]
    concat_zeros = [np.zeros((NCORES * s[0], *s[1:]), dt)
                    for (s, dt) in out_shapes]
    out_arrs = fn(*concat_in, *concat_zeros)
    res = []
    for c in range(NCORES):
        res.append({name: np.asarray(out_arrs[i]).reshape(
            NCORES, *out_shapes[i][0])[c] for i, name in enumerate(out_names)})
    return res


def make_in_maps(h, seg_ids, conf, W1, b1, W2, b2):
    h = np.ascontiguousarray(np.asarray(h), dtype=np.float32)
    seg_ids = np.ascontiguousarray(np.asarray(seg_ids), dtype=np.int32)
    conf = np.ascontiguousarray(np.asarray(conf), dtype=np.int32)
    W1 = np.ascontiguousarray(np.asarray(W1), dtype=np.float32)
    b1 = np.ascontiguousarray(np.asarray(b1), dtype=np.float32)
    W2 = np.ascontiguousarray(np.asarray(W2), dtype=np.float32)
    b2 = np.ascontiguousarray(np.asarray(b2), dtype=np.float32)
    in_maps = []
    for i in range(NCORES):
        sl = slice(i * BPC, (i + 1) * BPC)
        in_maps.append({
            "h": h[sl].reshape(BPC * S, D),
            "seg": seg_ids[sl],
            "conf": conf[sl],
            "w1": W1, "b1": b1, "w2": W2, "b2": b2,
        })
    return in_maps


def run(in_maps, trace=False, **kwargs):
    nc = _get_nc()
    return run_bass_kernel_spmd(nc, in_maps, core_ids=list(range(NCORES)),
                                trace=trace, **kwargs)


def kernel(h, seg_ids, conf, W1, b1, W2, b2):
    global _RUNNER
    in_maps = make_in_maps(h, seg_ids, conf, W1, b1, W2, b2)
    # The axon-tunneled devices occasionally fail the first execution after a
    # fresh NEFF load (NRT_EXEC_UNIT_UNRECOVERABLE); a retry on a rebuilt
    # executable has always succeeded. Guard the graded call.
    last = None
    for attempt in range(3):
        try:
            res = run_cached(in_maps)
            break
        except Exception as e:  # noqa: BLE001
            last = e
            _RUNNER = None
            import time as _time
            _time.sleep(2.0 * (attempt + 1))
    else:
        raise last
    outs = [res[i]["out"] for i in range(NCORES)]
    return np.concatenate(outs, axis=0)
